# revision 14
# baseline (speedup 1.0000x reference)
"""Multi-head self-attention TRN2 Bass kernel (8-core SPMD).

Problem: z [4, 2048, 1024], w_q/w_k/w_v/w_o [1024, 1024] (torch Linear
convention: q = z @ w_q.T), b_o [1024]. 16 heads x 64 dims, softmax scale
1/sqrt(64).

Sharding: 8 cores = (4 batches) x (2 query-halves). Each core computes full
K/V for its batch (projection duplicated 2x across the query-half pair) and
attention + output projection for its 1024 queries. No collectives; host
concatenates per-core outputs.

Device-side layout: everything is computed transposed (contraction dim on
partitions). Host feeds z[b].T with the core's query tokens permuted to the
END of the token axis, so the query slice is a fixed (SPMD-identical) column
range. Softmax is unnormalized flash-style: exp(S) streams straight into the
AV matmul whose stationary operand carries an extra all-ones column that
accumulates the denominators; normalization happens at PSUM eviction.
V (augmented) and O.T round-trip through DRAM scratch to fit SBUF.
"""

import os
import sys

import numpy as np

for _p in ("/opt/trn_rl_repo", "/root/.axon_site/_ro/trn_rl_repo"):
    if os.path.isdir(_p) and _p not in sys.path:
        sys.path.insert(0, _p)

import concourse.bacc as bacc
import concourse.mybir as mybir
import concourse.tile as tile
from concourse import bass_utils

F32 = mybir.dt.float32
F32R = mybir.dt.float32r
P = 128


def full_cfg():
    return dict(EMB=1024, N=2048, NQ=1024, H=16, DH=64)


def small_cfg():
    return dict(EMB=256, N=256, NQ=128, H=4, DH=64)


def build_program(nc, cfg):
    EMB, N, NQ, H, DH = cfg["EMB"], cfg["N"], cfg["NQ"], cfg["H"], cfg["DH"]
    EC = EMB // P            # emb contraction chunks
    TC = N // P              # key-token chunks
    PAIRS = H // 2           # head pairs (128 dims each)
    EMBH = EMB // 2          # V computed in two dout halves
    H2 = EMBH // DH          # heads per V half
    QB = min(512, NQ)        # query block (matmul moving size)
    NQB = NQ // QB
    PW = min(512, EMBH)      # V psum width
    SCALE = 1.0 / np.sqrt(DH)
    DA = DH + 1              # V head dims + ones column

    zt_d = nc.dram_tensor("zt", [EMB, N], F32R, kind="ExternalInput").ap()
    wq_d = nc.dram_tensor("wq", [EMB, EMB], F32R, kind="ExternalInput").ap()
    wk_d = nc.dram_tensor("wk", [EMB, EMB], F32R, kind="ExternalInput").ap()
    wv_d = nc.dram_tensor("wv", [EMB, EMB], F32R, kind="ExternalInput").ap()
    wo_d = nc.dram_tensor("wo", [EMB, EMB], F32R, kind="ExternalInput").ap()
    bo_d = nc.dram_tensor("bo", [EMB], F32, kind="ExternalInput").ap()
    yt_d = nc.dram_tensor("yt", [EMB, NQ], F32, kind="ExternalOutput").ap()

    def rearr(ap):  # [EMB, X] dram -> [P, EC, X] partition view
        return ap.rearrange("(eo p) x -> p eo x", p=P)

    # ST head slices sit at a fixed 512 stride so the two row-packed matmuls
    # always drain into different PSUM banks.
    BIGW = max(PW, min(1024, NQ), min(1024, N), 512 + QB)

    with tile.TileContext(nc) as tc:
        with (
            tc.tile_pool(name="dram", bufs=1, space="DRAM") as dram,
            tc.tile_pool(name="const", bufs=1) as const,
            tc.tile_pool(name="wvp", bufs=1) as wvp,
            tc.tile_pool(name="stg", bufs=2) as stg,
            tc.tile_pool(name="attn", bufs=2) as attn,
            tc.tile_pool(name="expp", bufs=3) as expp,
            tc.tile_pool(name="tmp", bufs=2) as tmp,
        ):
            vaug_dram = dram.tile([TC, P, H, DA], F32R)   # V + ones column
            ot_dram = dram.tile([EMB, NQ], F32R)          # normalized O.T

            zt_t = []
            for _ec in range(EC):
                zte = const.tile([P, N], F32R, tag=f"zt{_ec}", name=f"zt{_ec}")
                nc.sync.dma_start(zte[:], rearr(zt_d)[:, _ec, :])
                zt_t.append(zte)
            bo_sb = const.tile([P, EC], F32)
            nc.sync.dma_start(bo_sb[:], bo_d.rearrange("(mo p) -> p mo", p=P))
            ones_row = const.tile([1, DH], F32R)
            nc.any.memset(ones_row[:].bitcast(F32), 1.0)
            REP = cfg.get("REP", 1)

            def emit_v_half(hf):
                wv_t = []
                for _ec in range(EC):
                    wve = wvp.tile([P, EMBH], F32R, tag=f"wv{_ec}", name=f"wv{_ec}")
                    nc.sync.dma_start(wve[:], rearr(wv_d)[:, _ec, hf * EMBH : (hf + 1) * EMBH])
                    wv_t.append(wve)
                for tci in range(TC):
                    for pwb in range(EMBH // PW):
                        ps = big_ps.tile([P, BIGW], F32, tag="big")
                        for ec in range(EC):
                            nc.tensor.matmul(
                                ps[:, :PW],
                                lhsT=zt_t[ec][:, tci * P : (tci + 1) * P],
                                rhs=wv_t[ec][:, pwb * PW : (pwb + 1) * PW],
                                start=(ec == 0),
                                stop=(ec == EC - 1),
                            )
                        nh = PW // DH
                        h0 = hf * H2 + pwb * nh
                        vs = stg.tile([P, nh, DA], F32R, tag="vs")
                        nc.vector.tensor_copy(
                            vs[:, :, 0:DH],
                            ps[:, :PW].rearrange("p (h d) -> p h d", d=DH),
                        )
                        nc.any.memset(vs[:, :, DH:DA].bitcast(F32), 1.0)
                        nc.sync.dma_start(vaug_dram[tci, :, h0 : h0 + nh, :], vs[:])

            def emit_kq(pair):
                wk_t = attn.tile([P, EC, P], F32R, tag="wk")
                nc.sync.dma_start(wk_t[:], rearr(wk_d)[:, :, pair * P : (pair + 1) * P])
                kt = attn.tile([P, N], F32R, tag="kt")
                RW = min(1024, N)
                for rnd in range(N // RW):
                    ps = big_ps.tile([P, BIGW], F32, tag="big")
                    for ec in range(EC):
                        for nb in range(RW // min(512, RW)):
                            w = min(512, RW)
                            nc.tensor.matmul(
                                ps[:, nb * w : (nb + 1) * w],
                                lhsT=wk_t[:, ec, :],
                                rhs=zt_t[ec][:, rnd * RW + nb * w : rnd * RW + (nb + 1) * w],
                                start=(ec == 0),
                                stop=(ec == EC - 1),
                            )
                    nc.vector.tensor_copy(kt[:, rnd * RW : (rnd + 1) * RW], ps[:, :RW])

                wq_t = attn.tile([P, EC, P], F32R, tag="wq")
                nc.sync.dma_start(wq_t[:], rearr(wq_d)[:, :, pair * P : (pair + 1) * P])
                qt = attn.tile([P, NQ], F32R, tag="qt")
                ps = big_ps.tile([P, BIGW], F32, tag="big")
                for ec in range(EC):
                    for nb in range(NQB):
                        nc.tensor.matmul(
                            ps[:, nb * QB : (nb + 1) * QB],
                            lhsT=wq_t[:, ec, :],
                            rhs=zt_t[ec][:, N - NQ + nb * QB : N - NQ + (nb + 1) * QB],
                            start=(ec == 0),
                            stop=(ec == EC - 1),
                        )
                nc.vector.tensor_copy(qt[:], ps[:, :NQ])
                return kt, qt

            def emit_attention(pair, kt, qt):
                vp = attn.tile([P, TC, 2, DA], F32R, tag="vp")
                nc.sync.dma_start(
                    vp[:],
                    vaug_dram[:, :, 2 * pair : 2 * pair + 2, :].rearrange(
                        "t p h a -> p t h a"
                    ),
                )
                for qb in range(NQB):
                    avs = []
                    for _hh in range(2):
                        av_t = av_ps.tile([DA, QB], F32, tag="av", name=f"av{_hh}")
                        avs.append(av_t)

                    def emit_av(kc, ex):
                        for hh in range(2):
                            nc.tensor.matmul(
                                avs[hh][:],
                                lhsT=vp[:, kc, hh, :],
                                rhs=ex[:, hh * QB : (hh + 1) * QB],
                                start=(kc == 0),
                                stop=(kc == TC - 1),
                            )

                    # AV for chunk kc-1 is emitted AFTER ST of chunk kc so the
                    # PE never head-of-line blocks on the exp of the current
                    # chunk: PE does ST(kc+1) while ACT runs exp(kc).
                    pend = None
                    for kc in range(TC):
                        st = big_ps.tile([P, BIGW], F32, tag="big")
                        for hh in range(2):
                            nc.tensor.matmul(
                                st[:, hh * 512 : hh * 512 + QB],
                                lhsT=kt[hh * DH : (hh + 1) * DH, kc * P : (kc + 1) * P],
                                rhs=qt[hh * DH : (hh + 1) * DH, qb * QB : (qb + 1) * QB],
                                start=True,
                                stop=True,
                                tile_position=(hh * DH, 0),
                            )
                        ex = expp.tile([P, 2 * QB], F32R, tag="ex")
                        if QB == 512:
                            nc.scalar.activation(
                                ex[:],
                                st[:, :1024],
                                mybir.ActivationFunctionType.Exp,
                                scale=float(SCALE),
                            )
                        else:
                            for hh in range(2):
                                nc.scalar.activation(
                                    ex[:, hh * QB : (hh + 1) * QB],
                                    st[:, hh * 512 : hh * 512 + QB],
                                    mybir.ActivationFunctionType.Exp,
                                    scale=float(SCALE),
                                )
                        if pend is not None:
                            emit_av(*pend)
                        pend = (kc, ex)
                    emit_av(*pend)
                    for hh in range(2):
                        h = 2 * pair + hh
                        d0 = tmp.tile([1, QB], F32R, tag="d0")
                        nc.vector.tensor_copy(d0[:], avs[hh][DH : DH + 1, :])
                        bc = big_ps.tile([P, BIGW], F32, tag="big")
                        nc.tensor.matmul(
                            bc[0:DH, 0:QB], lhsT=ones_row[:], rhs=d0[:],
                            start=True, stop=True,
                        )
                        dn = tmp.tile([DH, QB], F32, tag="dn")
                        nc.vector.reciprocal(dn[:], bc[0:DH, 0:QB])
                        onr = tmp.tile([DH, QB], F32R, tag="onr")
                        nc.vector.tensor_mul(onr[:], avs[hh][0:DH, :], dn[:])
                        nc.sync.dma_start(
                            ot_dram[h * DH : (h + 1) * DH, qb * QB : (qb + 1) * QB],
                            onr[:],
                        )

            for _rep in range(REP):
              with (
                tc.tile_pool(name="big_ps", bufs=3, space="PSUM") as big_ps,
                tc.tile_pool(name="av_ps", bufs=2, space="PSUM") as av_ps,
              ):
                emit_v_half(0)
                for pair in range(PAIRS):
                    if pair == PAIRS // 2:
                        emit_v_half(1)
                    kt, qt = emit_kq(pair)
                    emit_attention(pair, kt, qt)

              # output projection: yt[m*128+j, q] = sum_e wo.T[e, m*128+j]*ot[e, q] + bo
              with tc.tile_pool(name="op_ps", bufs=EC, space="PSUM") as op_ps:
                OPW = min(512, NQ)
                wo_sb = const.tile([P, EC, EMB], F32R, tag="wo_sb", name="wo_sb")
                nc.sync.dma_start(wo_sb[:], rearr(wo_d))
                for qhb in range(NQ // OPW):
                    pss = []
                    for m in range(EC):
                        ps_t = op_ps.tile([P, OPW], F32, tag="op", name=f"op{m}")
                        pss.append(ps_t)
                    for ec in range(EC):
                        otc = stg.tile([P, OPW], F32R, tag="otc")
                        nc.sync.dma_start(
                            otc[:],
                            rearr(ot_dram[:])[:, ec, qhb * OPW : (qhb + 1) * OPW],
                        )
                        for m in range(EC):
                            nc.tensor.matmul(
                                pss[m][:],
                                lhsT=wo_sb[:, ec, m * P : (m + 1) * P],
                                rhs=otc[:],
                                start=(ec == 0),
                                stop=(ec == EC - 1),
                            )
                    for m in range(EC):
                        yt_t = tmp.tile([P, OPW], F32, tag="yt")
                        nc.vector.tensor_scalar(
                            yt_t[:], pss[m][:], bo_sb[:, m : m + 1], None,
                            op0=mybir.AluOpType.add,
                        )
                        nc.sync.dma_start(
                            yt_d[m * P : (m + 1) * P, qhb * OPW : (qhb + 1) * OPW],
                            yt_t[:],
                        )

    return nc


_COMPILED = {}


def get_compiled(cfg_name="full"):
    if cfg_name not in _COMPILED:
        cfg = full_cfg() if cfg_name == "full" else small_cfg()
        nc = bacc.Bacc("TRN2", target_bir_lowering=False, debug=False, num_devices=1)
        build_program(nc, cfg)
        nc.compile()
        _COMPILED[cfg_name] = nc
    return _COMPILED[cfg_name]


def make_in_maps(z, w_q, w_k, w_v, w_o, b_o):
    """Host-side shard: 8 cores = (batch, query-half). Query tokens are
    permuted to the end of the token axis so the query slice is SPMD-fixed."""
    B, N, EMB = z.shape
    NQ = N // 2
    wqT = np.ascontiguousarray(w_q.T.astype(np.float32))
    wkT = np.ascontiguousarray(w_k.T.astype(np.float32))
    wvT = np.ascontiguousarray(w_v.T.astype(np.float32))
    woT = np.ascontiguousarray(w_o.T.astype(np.float32))
    bo = np.ascontiguousarray(b_o.astype(np.float32))
    in_maps = []
    for c in range(8):
        b, qh = c // 2, c % 2
        zT = z[b].T.astype(np.float32)  # [EMB, N]
        if qh == 0:
            zp = np.concatenate([zT[:, NQ:], zT[:, :NQ]], axis=1)
        else:
            zp = zT
        in_maps.append(
            {"zt": np.ascontiguousarray(zp), "wq": wqT, "wk": wkT, "wv": wvT,
             "wo": woT, "bo": bo}
        )
    return in_maps


def kernel(z, w_q, w_k, w_v, w_o, b_o):
    B, N, EMB = z.shape
    NQ = N // 2
    nc = get_compiled("full")
    in_maps = make_in_maps(z, w_q, w_k, w_v, w_o, b_o)
    res = bass_utils.run_bass_kernel_spmd(nc, in_maps, core_ids=list(range(8)))
    y = np.empty((B, N, EMB), dtype=np.float32)
    for c in range(8):
        b, qh = c // 2, c % 2
        y[b, qh * NQ : (qh + 1) * NQ, :] = res.results[c]["yt"].T
    return y


# revision 15
# speedup vs baseline: 1.0644x; 1.0644x over previous
"""Multi-head self-attention TRN2 Bass kernel (8-core SPMD).

Problem: z [4, 2048, 1024], w_q/w_k/w_v/w_o [1024, 1024] (torch Linear
convention: q = z @ w_q.T), b_o [1024]. 16 heads x 64 dims, softmax scale
1/sqrt(64).

Sharding: 8 cores = (4 batches) x (2 query-halves). Each core computes full
K/V for its batch (projection duplicated 2x across the query-half pair) and
attention + output projection for its 1024 queries. No collectives; host
concatenates per-core outputs.

Device-side layout: everything is computed transposed (contraction dim on
partitions). Host feeds z[b].T with the core's query tokens permuted to the
END of the token axis, so the query slice is a fixed (SPMD-identical) column
range. Softmax is unnormalized flash-style: exp(S) streams straight into the
AV matmul whose stationary operand carries an extra all-ones column that
accumulates the denominators; normalization happens at PSUM eviction.
V (augmented) and O.T round-trip through DRAM scratch to fit SBUF.
"""

import os
import sys

import numpy as np

for _p in ("/opt/trn_rl_repo", "/root/.axon_site/_ro/trn_rl_repo"):
    if os.path.isdir(_p) and _p not in sys.path:
        sys.path.insert(0, _p)

import concourse.bacc as bacc
import concourse.mybir as mybir
import concourse.tile as tile
from concourse import bass_utils

F32 = mybir.dt.float32
F32R = mybir.dt.float32r
P = 128


def full_cfg():
    return dict(EMB=1024, N=2048, NQ=1024, H=16, DH=64)


def small_cfg():
    return dict(EMB=256, N=256, NQ=128, H=4, DH=64)


def build_program(nc, cfg):
    EMB, N, NQ, H, DH = cfg["EMB"], cfg["N"], cfg["NQ"], cfg["H"], cfg["DH"]
    EC = EMB // P            # emb contraction chunks
    TC = N // P              # key-token chunks
    PAIRS = H // 2           # head pairs (128 dims each)
    EMBH = EMB // 2          # V computed in two dout halves
    H2 = EMBH // DH          # heads per V half
    QB = min(512, NQ)        # query block (matmul moving size)
    NQB = NQ // QB
    PW = min(512, EMBH)      # V psum width
    SCALE = 1.0 / np.sqrt(DH)
    DA = DH + 1              # V head dims + ones column

    zt_d = nc.dram_tensor("zt", [EMB, N], F32R, kind="ExternalInput").ap()
    wq_d = nc.dram_tensor("wq", [EMB, EMB], F32R, kind="ExternalInput").ap()
    wk_d = nc.dram_tensor("wk", [EMB, EMB], F32R, kind="ExternalInput").ap()
    wv_d = nc.dram_tensor("wv", [EMB, EMB], F32R, kind="ExternalInput").ap()
    wo_d = nc.dram_tensor("wo", [EMB, EMB], F32R, kind="ExternalInput").ap()
    bo_d = nc.dram_tensor("bo", [EMB], F32, kind="ExternalInput").ap()
    yt_d = nc.dram_tensor("yt", [EMB, NQ], F32, kind="ExternalOutput").ap()

    def rearr(ap):  # [EMB, X] dram -> [P, EC, X] partition view
        return ap.rearrange("(eo p) x -> p eo x", p=P)

    # ST head slices sit at a fixed 512 stride so the two row-packed matmuls
    # always drain into different PSUM banks.
    BIGW = max(PW, min(1024, NQ), min(1024, N), 512 + QB)

    with tile.TileContext(nc) as tc:
        with (
            tc.tile_pool(name="dram", bufs=1, space="DRAM") as dram,
            tc.tile_pool(name="const", bufs=1) as const,
            tc.tile_pool(name="wvp", bufs=1) as wvp,
            tc.tile_pool(name="stg", bufs=2) as stg,
            tc.tile_pool(name="attn", bufs=2) as attn,
            tc.tile_pool(name="expp", bufs=3) as expp,
            tc.tile_pool(name="tmp", bufs=2) as tmp,
        ):
            vaug_dram = dram.tile([TC, P, H, DA], F32R)   # V + ones column
            ot_dram = dram.tile([EMB, NQ], F32R)          # normalized O.T

            zt_t = []
            for _ec in range(EC):
                zte = const.tile([P, N], F32R, tag=f"zt{_ec}", name=f"zt{_ec}")
                nc.sync.dma_start(zte[:], rearr(zt_d)[:, _ec, :])
                zt_t.append(zte)
            bo_sb = const.tile([P, EC], F32)
            nc.sync.dma_start(bo_sb[:], bo_d.rearrange("(mo p) -> p mo", p=P))
            ones_row = const.tile([1, DH], F32R)
            nc.any.memset(ones_row[:].bitcast(F32), 1.0)
            REP = cfg.get("REP", 1)

            def emit_v_half(hf):
                wv_t = []
                for _ec in range(EC):
                    wve = wvp.tile([P, EMBH], F32R, tag=f"wv{_ec}", name=f"wv{_ec}")
                    nc.sync.dma_start(wve[:], rearr(wv_d)[:, _ec, hf * EMBH : (hf + 1) * EMBH])
                    wv_t.append(wve)
                for tci in range(TC):
                    for pwb in range(EMBH // PW):
                        ps = big_ps.tile([P, BIGW], F32, tag="big")
                        for ec in range(EC):
                            nc.tensor.matmul(
                                ps[:, :PW],
                                lhsT=zt_t[ec][:, tci * P : (tci + 1) * P],
                                rhs=wv_t[ec][:, pwb * PW : (pwb + 1) * PW],
                                start=(ec == 0),
                                stop=(ec == EC - 1),
                            )
                        nh = PW // DH
                        h0 = hf * H2 + pwb * nh
                        vs = stg.tile([P, nh, DA], F32R, tag="vs")
                        nc.vector.tensor_copy(
                            vs[:, :, 0:DH],
                            ps[:, :PW].rearrange("p (h d) -> p h d", d=DH),
                        )
                        nc.any.memset(vs[:, :, DH:DA].bitcast(F32), 1.0)
                        nc.sync.dma_start(vaug_dram[tci, :, h0 : h0 + nh, :], vs[:])

            def emit_kq(pair):
                wk_t = attn.tile([P, EC, P], F32R, tag="wk")
                nc.sync.dma_start(wk_t[:], rearr(wk_d)[:, :, pair * P : (pair + 1) * P])
                kt = attn.tile([P, N], F32R, tag="kt")
                RW = min(1024, N)
                for rnd in range(N // RW):
                    ps = big_ps.tile([P, BIGW], F32, tag="big")
                    for ec in range(EC):
                        for nb in range(RW // min(512, RW)):
                            w = min(512, RW)
                            nc.tensor.matmul(
                                ps[:, nb * w : (nb + 1) * w],
                                lhsT=wk_t[:, ec, :],
                                rhs=zt_t[ec][:, rnd * RW + nb * w : rnd * RW + (nb + 1) * w],
                                start=(ec == 0),
                                stop=(ec == EC - 1),
                            )
                    nc.vector.tensor_copy(kt[:, rnd * RW : (rnd + 1) * RW], ps[:, :RW])

                wq_t = attn.tile([P, EC, P], F32R, tag="wq")
                nc.sync.dma_start(wq_t[:], rearr(wq_d)[:, :, pair * P : (pair + 1) * P])
                qt = attn.tile([P, NQ], F32R, tag="qt")
                ps = big_ps.tile([P, BIGW], F32, tag="big")
                for ec in range(EC):
                    for nb in range(NQB):
                        nc.tensor.matmul(
                            ps[:, nb * QB : (nb + 1) * QB],
                            lhsT=wq_t[:, ec, :],
                            rhs=zt_t[ec][:, N - NQ + nb * QB : N - NQ + (nb + 1) * QB],
                            start=(ec == 0),
                            stop=(ec == EC - 1),
                        )
                nc.vector.tensor_copy(qt[:], ps[:, :NQ])
                return kt, qt

            def emit_attention(pair, kt, qt):
                vp = attn.tile([P, TC, 2, DA], F32R, tag="vp")
                nc.sync.dma_start(
                    vp[:],
                    vaug_dram[:, :, 2 * pair : 2 * pair + 2, :].rearrange(
                        "t p h a -> p t h a"
                    ),
                )
                for qb in range(NQB):
                    avs = []
                    for _hh in range(2):
                        av_t = av_ps.tile([DA, QB], F32, tag="av", name=f"av{_hh}")
                        avs.append(av_t)

                    def emit_av(kc, ex):
                        for hh in range(2):
                            nc.tensor.matmul(
                                avs[hh][:],
                                lhsT=vp[:, kc, hh, :],
                                rhs=ex[:, hh * QB : (hh + 1) * QB],
                                start=(kc == 0),
                                stop=(kc == TC - 1),
                            )

                    # AV for chunk kc-1 is emitted AFTER ST of chunk kc so the
                    # PE never head-of-line blocks on the exp of the current
                    # chunk: PE does ST(kc+1) while ACT runs exp(kc).
                    pend = None
                    for kc in range(TC):
                        st = big_ps.tile([P, BIGW], F32, tag="big")
                        for hh in range(2):
                            nc.tensor.matmul(
                                st[:, hh * 512 : hh * 512 + QB],
                                lhsT=kt[hh * DH : (hh + 1) * DH, kc * P : (kc + 1) * P],
                                rhs=qt[hh * DH : (hh + 1) * DH, qb * QB : (qb + 1) * QB],
                                start=True,
                                stop=True,
                                tile_position=(hh * DH, 0),
                            )
                        ex = expp.tile([P, 2 * QB], F32R, tag="ex")
                        if QB == 512:
                            nc.scalar.activation(
                                ex[:],
                                st[:, :1024],
                                mybir.ActivationFunctionType.Exp,
                                scale=float(SCALE),
                            )
                        else:
                            for hh in range(2):
                                nc.scalar.activation(
                                    ex[:, hh * QB : (hh + 1) * QB],
                                    st[:, hh * 512 : hh * 512 + QB],
                                    mybir.ActivationFunctionType.Exp,
                                    scale=float(SCALE),
                                )
                        if pend is not None:
                            emit_av(*pend)
                        pend = (kc, ex)
                    emit_av(*pend)
                    for hh in range(2):
                        h = 2 * pair + hh
                        d0 = tmp.tile([1, QB], F32, tag="d0")
                        nc.vector.tensor_copy(d0[:], avs[hh][DH : DH + 1, :])
                        dn = tmp.tile([DH, QB], F32, tag="dn")
                        nc.gpsimd.partition_broadcast(dn[:], d0[:])
                        nc.vector.reciprocal(dn[:], dn[:])
                        onr = tmp.tile([DH, QB], F32R, tag="onr")
                        nc.vector.tensor_mul(onr[:], avs[hh][0:DH, :], dn[:])
                        nc.sync.dma_start(
                            ot_dram[h * DH : (h + 1) * DH, qb * QB : (qb + 1) * QB],
                            onr[:],
                        )

            for _rep in range(REP):
              with (
                tc.tile_pool(name="big_ps", bufs=2, space="PSUM") as big_ps,
                tc.tile_pool(name="av_ps", bufs=4, space="PSUM") as av_ps,
              ):
                emit_v_half(0)
                for pair in range(PAIRS):
                    if pair == PAIRS // 2:
                        emit_v_half(1)
                    kt, qt = emit_kq(pair)
                    emit_attention(pair, kt, qt)

              # output projection: yt[m*128+j, q] = sum_e wo.T[e, m*128+j]*ot[e, q] + bo
              with tc.tile_pool(name="op_ps", bufs=EC, space="PSUM") as op_ps:
                OPW = min(512, NQ)
                wo_sb = const.tile([P, EC, EMB], F32R, tag="wo_sb", name="wo_sb")
                nc.sync.dma_start(wo_sb[:], rearr(wo_d))
                for qhb in range(NQ // OPW):
                    pss = []
                    for m in range(EC):
                        ps_t = op_ps.tile([P, OPW], F32, tag="op", name=f"op{m}")
                        pss.append(ps_t)
                    for ec in range(EC):
                        otc = stg.tile([P, OPW], F32R, tag="otc")
                        nc.sync.dma_start(
                            otc[:],
                            rearr(ot_dram[:])[:, ec, qhb * OPW : (qhb + 1) * OPW],
                        )
                        for m in range(EC):
                            nc.tensor.matmul(
                                pss[m][:],
                                lhsT=wo_sb[:, ec, m * P : (m + 1) * P],
                                rhs=otc[:],
                                start=(ec == 0),
                                stop=(ec == EC - 1),
                            )
                    for m in range(EC):
                        yt_t = tmp.tile([P, OPW], F32, tag="yt")
                        nc.vector.tensor_scalar(
                            yt_t[:], pss[m][:], bo_sb[:, m : m + 1], None,
                            op0=mybir.AluOpType.add,
                        )
                        nc.sync.dma_start(
                            yt_d[m * P : (m + 1) * P, qhb * OPW : (qhb + 1) * OPW],
                            yt_t[:],
                        )

    return nc


_COMPILED = {}


def get_compiled(cfg_name="full"):
    if cfg_name not in _COMPILED:
        cfg = full_cfg() if cfg_name == "full" else small_cfg()
        nc = bacc.Bacc("TRN2", target_bir_lowering=False, debug=False, num_devices=1)
        build_program(nc, cfg)
        nc.compile()
        _COMPILED[cfg_name] = nc
    return _COMPILED[cfg_name]


def make_in_maps(z, w_q, w_k, w_v, w_o, b_o):
    """Host-side shard: 8 cores = (batch, query-half). Query tokens are
    permuted to the end of the token axis so the query slice is SPMD-fixed."""
    B, N, EMB = z.shape
    NQ = N // 2
    wqT = np.ascontiguousarray(w_q.T.astype(np.float32))
    wkT = np.ascontiguousarray(w_k.T.astype(np.float32))
    wvT = np.ascontiguousarray(w_v.T.astype(np.float32))
    woT = np.ascontiguousarray(w_o.T.astype(np.float32))
    bo = np.ascontiguousarray(b_o.astype(np.float32))
    in_maps = []
    for c in range(8):
        b, qh = c // 2, c % 2
        zT = z[b].T.astype(np.float32)  # [EMB, N]
        if qh == 0:
            zp = np.concatenate([zT[:, NQ:], zT[:, :NQ]], axis=1)
        else:
            zp = zT
        in_maps.append(
            {"zt": np.ascontiguousarray(zp), "wq": wqT, "wk": wkT, "wv": wvT,
             "wo": woT, "bo": bo}
        )
    return in_maps


def kernel(z, w_q, w_k, w_v, w_o, b_o):
    B, N, EMB = z.shape
    NQ = N // 2
    nc = get_compiled("full")
    in_maps = make_in_maps(z, w_q, w_k, w_v, w_o, b_o)
    res = bass_utils.run_bass_kernel_spmd(nc, in_maps, core_ids=list(range(8)))
    y = np.empty((B, N, EMB), dtype=np.float32)
    for c in range(8):
        b, qh = c // 2, c % 2
        y[b, qh * NQ : (qh + 1) * NQ, :] = res.results[c]["yt"].T
    return y
